# revision 8
# baseline (speedup 1.0000x reference)
"""Cross-attention layer (B=2, T=2048, C=3072, D=1024, 16 heads x 64) on 8 TRN2 cores.

Sharding: batch (2) x head-groups (4). Core i handles batch b=i//4 and the 4
heads [4*(i%4), 4*(i%4)+4). Q/K/V projections are column-sharded by head,
the output projection is row-sharded, so each core returns a partial [T, D]
output (bf16); the host sums the 4 partials per batch element and adds bo.

On-core dataflow (bf16 matmuls, fp32 PSUM accum):
  phase A: qT[256,2048], then V -> PE-transpose -> v[c,260] (65 cols/head:
  64 v + ones col from a memset, not DMA).
  attention runs per head-PAIR (2p, 2p+1): the two heads' QK matmuls are
  row-tiled at PE positions (0,0)/(64,0) and issued back-to-back ABAB per
  512-col half; measured on hw, an ABAB pair of 64-contraction matmuls
  streams concurrently (~216ns/pair vs ~432 serial), recovering the full
  128-row array rate for the d=64 attention contraction. exp on ACT
  (scale=1/8 fused); PV accumulates uo_ps[65, t] per head (row 64 = softmax
  denominator via a ones column in v).
  The K projection (pairs 1-2) and first half of the output projection
  (pairs 3-4) are interleaved INTO the attention ci-loop as just-in-time
  "filler" blocks; the ci loop itself is ACT-bound (2x 1147ns exp per ci),
  so fillers ride in the PE headroom.
  normalize: denominators broadcast via mask-matmul, reciprocal+mul on DVE
  out[t, D] = sum_p uo_p^T-slice @ WoT_shard    (bf16, accumulated in PSUM)
  DMA: weights ride one jumbo descriptor; input triggers are split across
  the two HWDGE queues (sync: w+x, scalar: iden+ctx+consts) so trigger
  serialization doesn't delay the first Q-projection matmuls.
"""
from collections import deque

import numpy as np
import concourse.bass as bass  # noqa: F401  (engine types re-exported via nc)
import concourse.mybir as mybir
import concourse.tile as tile
from concourse import bacc
from concourse.bass import ds, ts
from concourse.bass_utils import run_bass_kernel_spmd
import ml_dtypes

_bf16 = ml_dtypes.bfloat16

B, T, C, D = 2, 2048, 3072, 1024
NH, DH = 16, 64
NCORES = 8
HL = 4                # heads per core
DL = HL * DH          # 256 local projection dims
DHP = DH + 1          # 65: per-head v columns incl. ones column

F32 = mybir.dt.float32
BF16 = mybir.dt.bfloat16
AF = mybir.ActivationFunctionType

KC = D // 128         # 8 contraction chunks for projections
CC = C // 128         # 24 key tiles
TT = T // 128         # 16 query tiles


def _emit(nc, tc, io):
    xT, ctxT, wqkvT, woT, bqkv, msk, iden, out = io

    with (
        tc.sbuf_pool(name="persist", bufs=1) as pp,
        tc.sbuf_pool(name="wqkv", bufs=1) as wp,
        tc.sbuf_pool(name="stream", bufs=1) as sp,
    ):
        qT = [pp.tile([128, T], BF16, name=f"qT{p}") for p in range(2)]
        kT = [pp.tile([128, C], BF16, name=f"kT{p}") for p in range(2)]
        v = pp.tile([128, CC * HL * DHP], BF16, name="v")
        wo = [pp.tile([128, D], BF16, name=f"wo{p}") for p in range(2)]
        msk_sb = pp.tile([65, 128], BF16, name="msk_sb")
        iden_sb = pp.tile([128, 128], BF16, name="iden_sb")
        b_sb = [pp.tile([128, 3], F32, name=f"b{p}") for p in range(2)]
        # ones columns for the softmax denominators: engine memset, not DMA
        # (strided 2-byte-packet DMAs at the head of the input queue cost
        # ~40us of dead PE time).
        ones_view = v.rearrange("a (i c) -> a i c", c=DHP)[:, :, DH:DHP]
        nc.gpsimd.memset(ones_view, 1.0)

        # All weight chunks ride ONE jumbo DMA descriptor into a single
        # [128, KC*768] tile (k-chunk k at free offset k*768): one trigger
        # instead of 8 on the head-critical sync queue.
        w_all = wp.tile([128, KC * 3 * DL], BF16, name="w_all")
        w_sb = [w_all[:, ds(k * 3 * DL, 3 * DL)] for k in range(KC)]
        xs = [sp.tile([128, T], BF16, tag=f"x{k}", name=f"xs{k}") for k in range(KC)]
        cs = [sp.tile([128, C], BF16, tag=f"c{k}", name=f"cs{k}") for k in range(KC)]

        # Priority order on the sync HWDGE queue: w (one jumbo trigger) then
        # x then ctx — everything the projections block on, in need-order so
        # transfers don't compete with each other for HBM bandwidth. The
        # scalar HWDGE queue carries only the small constants concurrently.
        nc.sync.dma_start(
            out=w_all.rearrange("p (k c) -> p k c", k=KC),
            in_=wqkvT.rearrange("(k p) c -> p k c", k=KC),
        )
        for k in range(KC):
            nc.sync.dma_start(out=xs[k], in_=xT[ts(k, 128), :])
        for k in range(KC):
            nc.sync.dma_start(out=cs[k], in_=ctxT[ts(k, 128), :])
        nc.scalar.dma_start(out=iden_sb, in_=iden)
        for p in range(2):
            nc.scalar.dma_start(out=b_sb[p], in_=bqkv[ts(p, 128), :])
        nc.scalar.dma_start(out=msk_sb, in_=msk)
        for p in range(2):
            nc.scalar.dma_start(out=wo[p], in_=woT[ts(p, 128), :])

        bq_sb = [b_sb[p][:, 0:1] for p in range(2)]
        bk_sb = [b_sb[p][:, 1:2] for p in range(2)]
        bv_sb = [b_sb[p][:, 2:3] for p in range(2)]

        wq = [w_sb[k][:, 0:DL] for k in range(KC)]
        wk = [w_sb[k][:, DL : 2 * DL] for k in range(KC)]
        wv = [w_sb[k][:, 2 * DL : 3 * DL] for k in range(KC)]

        # ---------------- Phase A: Q and V projections ----------------
        # k-outer loops: all output PSUM groups live at once, so matmuls
        # issue as each input k-chunk's DMA lands instead of waiting for the
        # full tensor (PE saturated from the first chunk arrival).
        with tc.psum_pool(name="qpp", bufs=8) as qpp:
            qgr = [
                qpp.tile([128, 512], F32, tag="proj", name=f"qp{p}{tq}")
                for p in range(2)
                for tq in range(4)
            ]
            for k in range(KC):
                for i in range(8):
                    p, tq = i // 4, i % 4
                    nc.tensor.matmul(
                        qgr[i],
                        lhsT=wq[k][:, ts(p, 128)],
                        rhs=xs[k][:, ts(tq, 512)],
                        start=(k == 0),
                        stop=(k == KC - 1),
                    )
            for i in range(8):
                p, tq = i // 4, i % 4
                nc.vector.tensor_scalar_add(
                    out=qT[p][:, ts(tq, 512)], in0=qgr[i], scalar1=bq_sb[p]
                )

        # V projection; transposes ride the DMA XBAR (SBUF->SBUF transpose
        # engine) instead of PE matmul-transposes + DVE copies, freeing
        # ~10us of PE and ~11us of DVE on the phase-A critical path.
        # (K is interleaved into attention.)
        with (
            tc.sbuf_pool(name="vstage", bufs=4) as vp,
            tc.psum_pool(name="pps", bufs=6) as pps,
        ):
            for p in range(2):
                vgr = [
                    pps.tile([128, 512], F32, tag="proj", name=f"vp{p}{cq}")
                    for cq in range(6)
                ]
                for k in range(KC):
                    for cq in range(6):
                        nc.tensor.matmul(
                            vgr[cq],
                            lhsT=wv[k][:, ts(p, 128)],
                            rhs=cs[k][:, ts(cq, 512)],
                            start=(k == 0),
                            stop=(k == KC - 1),
                        )
                for cq in range(6):
                    vst = vp.tile([128, 512], BF16, tag="vst", name=f"vs{p}{cq}")
                    nc.vector.tensor_scalar_add(out=vst, in0=vgr[cq], scalar1=bv_sb[p])
                    for cb in range(4):
                        ci = cq * 4 + cb
                        tst = vp.tile([128, 128], BF16, tag="tst", name=f"tt{ci}{p}")
                        nc.sync.dma_start_transpose(tst, vst[:, ts(cb, 128)])
                        src_ = tst.rearrange("a (h c) -> a h c", h=2)
                        dst = v[:, ds(ci * HL * DHP + 2 * p * DHP, 2 * DHP)].rearrange(
                            "a (h c) -> a h c", h=2
                        )[:, :, 0:DH]
                        nc.vector.tensor_copy(out=dst, in_=src_)

        # ---------------- Phases B + C (K-proj + out-proj interleaved) ----
        with (
            tc.sbuf_pool(name="uop", bufs=1) as up,
            tc.sbuf_pool(name="rsp", bufs=1) as rp,
            tc.sbuf_pool(name="obp", bufs=3) as ob,
            tc.sbuf_pool(name="expp", bufs=3) as ep,
            tc.sbuf_pool(name="rcp", bufs=2) as rc,
            tc.psum_pool(name="qkps", bufs=2) as qps,
            tc.psum_pool(name="uops", bufs=2) as ups,
        ):
            uo = [up.tile([128, T], BF16, name=f"uo{pr}") for pr in range(2)]
            rs = [rp.tile([65, T], BF16, name=f"rs{pr}") for pr in range(2)]
            for pr in range(2):
                nc.gpsimd.memset(rs[pr], 1.0)

            def kproj_block(pp_, cq):
                def emit():
                    ps = qps.tile([128, 512], F32, tag="qk", name=f"kp{pp_}{cq}")
                    for k in range(KC):
                        nc.tensor.matmul(
                            ps,
                            lhsT=wk[k][:, ts(pp_, 128)],
                            rhs=cs[k][:, ts(cq, 512)],
                            start=(k == 0),
                            stop=(k == KC - 1),
                        )
                    nc.vector.tensor_scalar_add(
                        out=kT[pp_][:, ts(cq, 512)], in0=ps, scalar1=bk_sb[pp_]
                    )

                return emit

            def c_tile_half(tt, dc, osb_holder, on_act=False):
                # half an out-projection t-tile (one 512-wide dc chunk);
                # split so a single filler invocation stays under the per-ci
                # PE headroom while ACT streams the exp.
                def emit():
                    if dc == 0:
                        osb_holder["t"] = ob.tile(
                            [128, D], BF16, tag="ob", name=f"ob{tt}"
                        )
                    osb = osb_holder["t"]
                    o_ps = qps.tile([128, 512], F32, tag="qk", name=f"o{tt}_{dc}")
                    for p in range(2):
                        nc.tensor.matmul(
                            o_ps,
                            lhsT=uo[p][:, ts(tt, 128)],
                            rhs=wo[p][:, ts(dc, 512)],
                            start=(p == 0),
                            stop=(p == 1),
                        )
                    if on_act and dc == 1:
                        nc.scalar.copy(out=osb[:, ts(dc, 512)], in_=o_ps)
                    else:
                        nc.vector.tensor_copy(out=osb[:, ts(dc, 512)], in_=o_ps)
                    if dc == 1:
                        nc.sync.dma_start(out=out[ts(tt, 128), :], in_=osb)

                return emit

            def c_tile(tt, on_act=False):
                h = {}
                a = c_tile_half(tt, 0, h, on_act)
                b = c_tile_half(tt, 1, h, on_act)

                def emit():
                    a()
                    b()

                return emit

            def norm_half(pr, th, tq):
                # broadcast denominators -> [128, 512], recip, scale uo half
                def emit():
                    off = th * 1024 + tq * 512
                    bc = qps.tile([128, 512], F32, tag="qk", name=f"bc{pr}{th}{tq}")
                    nc.tensor.matmul(
                        bc,
                        lhsT=msk_sb,
                        rhs=rs[pr][:, ds(off, 512)],
                        start=True,
                        stop=True,
                    )
                    rcl = rc.tile([128, 512], F32, tag="rc", name=f"rcl{pr}{th}{tq}")
                    nc.vector.reciprocal_approx_fast(rcl, bc)
                    nc.vector.tensor_mul(
                        out=uo[pr][:, ds(off, 512)],
                        in0=uo[pr][:, ds(off, 512)],
                        in1=rcl,
                    )

                return emit

            def attn_pair(p, tb, pre, post):
                # heads (2p, 2p+1) together: per 512-col half the two heads'
                # QK matmuls are issued ABAB into PE row tiles (0,0)/(64,0);
                # adjacent issue makes the hardware stream them concurrently.
                # Per ci: QK + exp issue FIRST (keeps the ACT exp stream fed),
                # then the previous ci's PV, then fillers riding in the PE
                # headroom under the ~2.3us ACT slot.
                uo_ps = [
                    ups.tile([65, 1024], F32, tag="uo", name=f"up{p}{tb}_{h2}")
                    for h2 in range(2)
                ]

                def pv_step(ci, h2, ex):
                    h = 2 * p + h2
                    vsl = v[:, ds((ci * HL + h) * DHP, DHP)]
                    for tq in range(2):
                        nc.tensor.matmul(
                            uo_ps[h2][:, ts(tq, 512)],
                            lhsT=vsl,
                            rhs=ex[:, ts(tq, 512)],
                            start=(ci == 0),
                            stop=(ci == CC - 1),
                        )

                pend = deque()
                for ci in range(CC):
                    for f in pre.get(ci, ()):
                        f()
                    qks = [
                        qps.tile([128, 1024], F32, tag="qk", name=f"qk{p}{tb}_{ci}_{h2}")
                        for h2 in range(2)
                    ]
                    for tq in range(2):
                        for h2 in range(2):
                            base = 64 * h2
                            nc.tensor.matmul(
                                qks[h2][:, ts(tq, 512)],
                                lhsT=kT[p][ds(base, 64), ts(ci, 128)],
                                rhs=qT[p][ds(base, 64), ds(tb * 1024 + tq * 512, 512)],
                                start=True,
                                stop=True,
                                tile_position=(base, 0),
                            )
                    exs = []
                    for h2 in range(2):
                        ex = ep.tile(
                            [128, 1024], BF16, tag="exp", name=f"ex{p}{tb}_{ci}_{h2}"
                        )
                        nc.scalar.activation(ex, qks[h2], AF.Exp, scale=0.125)
                        exs.append(ex)
                    while pend:
                        pv_step(*pend.popleft())
                    for h2 in range(2):
                        pend.append((ci, h2, exs[h2]))
                    for f in post.get(ci, ()):
                        f()
                # the last PVs and the drain are NOT emitted here: they're
                # returned as carry closures for the next pair's ci0/ci1
                # filler slots, so the next QK stream overlaps the tail
                # exp-wait and the boundary never goes PE-quiet.
                last = list(pend)
                pend.clear()

                def carry_pv():
                    for it in last:
                        pv_step(*it)

                def carry_drain():
                    # keep ACT free for the exp stream: all drains on DVE
                    for h2 in range(2):
                        nc.vector.tensor_copy(
                            out=uo[p][ds(64 * h2, 64), ds(tb * 1024, 1024)],
                            in_=uo_ps[h2][0:64, :],
                        )
                        nc.vector.tensor_copy(
                            out=rs[p][ds(64 * h2, 1), ds(tb * 1024, 1024)],
                            in_=uo_ps[h2][64:65, :],
                        )

                return [carry_pv, carry_drain]

            # Norms and each pair's tail drain run as fillers inside the
            # NEXT pair (off the boundary critical path); K-proj feeds
            # pairs 1-2 just-in-time, one cq block ahead; out-projection
            # t-tiles 0-7 hide in pairs 3-4 as split halves.
            # pair 1 (p0,tb0): only ci0 uses pre-fillers (kT cq0-1 gate QK)
            cr = attn_pair(
                0,
                0,
                {0: [kproj_block(0, 0), kproj_block(0, 1)]},
                {
                    1: [kproj_block(0, 2)],
                    5: [kproj_block(0, 3)],
                    9: [kproj_block(0, 4)],
                    13: [kproj_block(0, 5)],
                    17: [kproj_block(1, 0)],
                    20: [kproj_block(1, 1)],
                    22: [kproj_block(1, 2)],
                },
            )
            # pair 2 (p1,tb0)
            cr = attn_pair(
                1,
                0,
                {},
                {
                    0: [cr[0]],
                    1: [cr[1]],
                    2: [kproj_block(1, 3)],
                    4: [norm_half(0, 0, 0)],
                    6: [norm_half(0, 0, 1)],
                    9: [kproj_block(1, 4)],
                    14: [kproj_block(1, 5)],
                },
            )
            ct_h = [dict() for _ in range(TT)]
            # pair 3 (p1,tb1)
            cr = attn_pair(
                1,
                1,
                {},
                {
                    0: [cr[0]],
                    1: [cr[1]],
                    2: [norm_half(1, 0, 0)],
                    3: [norm_half(1, 0, 1)],
                    4: [c_tile_half(0, 0, ct_h[0])],
                    6: [c_tile_half(0, 1, ct_h[0])],
                    9: [c_tile_half(1, 0, ct_h[1])],
                    11: [c_tile_half(1, 1, ct_h[1])],
                    14: [c_tile_half(2, 0, ct_h[2])],
                    16: [c_tile_half(2, 1, ct_h[2])],
                    19: [c_tile_half(3, 0, ct_h[3])],
                    21: [c_tile_half(3, 1, ct_h[3])],
                },
            )
            # pair 4 (p0,tb1)
            cr = attn_pair(
                0,
                1,
                {},
                {
                    0: [cr[0]],
                    1: [cr[1]],
                    2: [norm_half(1, 1, 0)],
                    3: [norm_half(1, 1, 1)],
                    4: [c_tile_half(4, 0, ct_h[4])],
                    6: [c_tile_half(4, 1, ct_h[4])],
                    9: [c_tile_half(5, 0, ct_h[5])],
                    11: [c_tile_half(5, 1, ct_h[5])],
                    14: [c_tile_half(6, 0, ct_h[6])],
                    16: [c_tile_half(6, 1, ct_h[6])],
                    19: [c_tile_half(7, 0, ct_h[7])],
                    21: [c_tile_half(7, 1, ct_h[7])],
                },
            )
            for f in cr:
                f()
            norm_half(0, 1, 0)()
            norm_half(0, 1, 1)()
            for tt in range(TT // 2, TT):
                c_tile(tt, on_act=True)()


def _build_nc():
    nc = bacc.Bacc("TRN2", target_bir_lowering=False, debug=False, num_devices=NCORES)
    xT = nc.dram_tensor("xT", [D, T], BF16, kind="ExternalInput").ap()
    ctxT = nc.dram_tensor("ctxT", [D, C], BF16, kind="ExternalInput").ap()
    wqkvT = nc.dram_tensor("wqkvT", [D, 3 * DL], BF16, kind="ExternalInput").ap()
    woT = nc.dram_tensor("woT", [DL, D], BF16, kind="ExternalInput").ap()
    bqkv = nc.dram_tensor("bqkv", [DL, 3], F32, kind="ExternalInput").ap()
    msk = nc.dram_tensor("msk", [65, 128], BF16, kind="ExternalInput").ap()
    iden = nc.dram_tensor("iden", [128, 128], BF16, kind="ExternalInput").ap()
    out = nc.dram_tensor("out", [T, D], BF16, kind="ExternalOutput").ap()
    with tile.TileContext(nc) as tc:
        _emit(nc, tc, (xT, ctxT, wqkvT, woT, bqkv, msk, iden, out))
    nc.compile()
    return nc


_NC_CACHE = None


def _get_nc():
    global _NC_CACHE
    if _NC_CACHE is None:
        _NC_CACHE = _build_nc()
    return _NC_CACHE


def _make_in_maps(inputs):
    x = np.asarray(inputs["x"], dtype=np.float32)
    context = np.asarray(inputs["context"], dtype=np.float32)
    Wq = np.asarray(inputs["Wq"], dtype=np.float32)
    Wk = np.asarray(inputs["Wk"], dtype=np.float32)
    Wv = np.asarray(inputs["Wv"], dtype=np.float32)
    Wo = np.asarray(inputs["Wo"], dtype=np.float32)
    bq = np.asarray(inputs["bq"], dtype=np.float32)
    bk = np.asarray(inputs["bk"], dtype=np.float32)
    bv = np.asarray(inputs["bv"], dtype=np.float32)

    msk = np.zeros((65, 128), _bf16)
    msk[0, :64] = 1.0
    msk[64, 64:] = 1.0
    iden = np.eye(128, dtype=_bf16)

    xTs = [np.ascontiguousarray(x[b].T).astype(_bf16) for b in range(B)]
    cTs = [np.ascontiguousarray(context[b].T).astype(_bf16) for b in range(B)]

    in_maps = []
    for core in range(NCORES):
        b, hg = core // 4, core % 4
        sl = slice(hg * DL, (hg + 1) * DL)
        in_maps.append(
            {
                "xT": xTs[b],
                "ctxT": cTs[b],
                "wqkvT": np.ascontiguousarray(
                    np.concatenate([Wq[sl].T, Wk[sl].T, Wv[sl].T], axis=1)
                ).astype(_bf16),
                "woT": np.ascontiguousarray(Wo[:, sl].T).astype(_bf16),
                "bqkv": np.ascontiguousarray(
                    np.stack([bq[sl], bk[sl], bv[sl]], axis=1)
                ),
                "msk": msk,
                "iden": iden,
            }
        )
    return in_maps


def run_spmd(inputs, trace=False):
    """Run the SPMD kernel; returns (full output [B,T,D], BassKernelResults)."""
    in_maps = _make_in_maps(inputs)
    res = run_bass_kernel_spmd(
        _get_nc(), in_maps, core_ids=list(range(NCORES)), trace=trace
    )
    bo = np.asarray(inputs["bo"], dtype=np.float32)
    y = np.zeros((B, T, D), np.float32)
    for core in range(NCORES):
        y[core // 4] += np.asarray(res.results[core]["out"], dtype=np.float32)
    y += bo.reshape(1, 1, D)
    return y, res


def kernel(**inputs):
    y, _ = run_spmd(inputs, trace=False)
    return y


# revision 13
# speedup vs baseline: 1.0163x; 1.0163x over previous
"""Cross-attention layer (B=2, T=2048, C=3072, D=1024, 16 heads x 64) on 8 TRN2 cores.

Sharding: batch (2) x head-groups (4). Core i handles batch b=i//4 and the 4
heads [4*(i%4), 4*(i%4)+4). Q/K/V projections are column-sharded by head,
the output projection is row-sharded, so each core returns a partial [T, D]
output (bf16); the host sums the 4 partials per batch element and adds bo.

On-core dataflow (bf16 matmuls, fp32 PSUM accum):
  phase A: qT[256,2048], then V -> PE-transpose -> v[c,260] (65 cols/head:
  64 v + ones col from a memset, not DMA).
  attention runs per head-PAIR (2p, 2p+1): the two heads' QK matmuls are
  row-tiled at PE positions (0,0)/(64,0) and issued back-to-back ABAB per
  512-col half; measured on hw, an ABAB pair of 64-contraction matmuls
  streams concurrently (~216ns/pair vs ~432 serial), recovering the full
  128-row array rate for the d=64 attention contraction. exp on ACT
  (scale=1/8 fused); PV accumulates uo_ps[65, t] per head (row 64 = softmax
  denominator via a ones column in v).
  The K projection (pairs 1-2) and first half of the output projection
  (pairs 3-4) are interleaved INTO the attention ci-loop as just-in-time
  "filler" blocks; the ci loop itself is ACT-bound (2x 1147ns exp per ci),
  so fillers ride in the PE headroom.
  normalize: denominators broadcast via mask-matmul, reciprocal+mul on DVE
  out[t, D] = sum_p uo_p^T-slice @ WoT_shard    (bf16, accumulated in PSUM)
  DMA: weights ride one jumbo descriptor; input triggers are split across
  the two HWDGE queues (sync: w+x, scalar: iden+ctx+consts) so trigger
  serialization doesn't delay the first Q-projection matmuls.
"""
from collections import deque

import numpy as np
import concourse.bass as bass  # noqa: F401  (engine types re-exported via nc)
import concourse.mybir as mybir
import concourse.tile as tile
from concourse import bacc
from concourse.bass import ds, ts
from concourse.bass_utils import run_bass_kernel_spmd
import ml_dtypes

_bf16 = ml_dtypes.bfloat16

B, T, C, D = 2, 2048, 3072, 1024
NH, DH = 16, 64
NCORES = 8
HL = 4                # heads per core
DL = HL * DH          # 256 local projection dims
DHP = DH + 1          # 65: per-head v columns incl. ones column

F32 = mybir.dt.float32
BF16 = mybir.dt.bfloat16
AF = mybir.ActivationFunctionType

KC = D // 128         # 8 contraction chunks for projections
CC = C // 128         # 24 key tiles
TT = T // 128         # 16 query tiles


def _emit(nc, tc, io):
    xT, ctxT, wqkvT, woT, bqkv, msk, iden, out = io

    with (
        tc.sbuf_pool(name="persist", bufs=1) as pp,
        tc.sbuf_pool(name="wqkv", bufs=1) as wp,
        tc.sbuf_pool(name="stream", bufs=1) as sp,
    ):
        qT = [pp.tile([128, T], BF16, name=f"qT{p}") for p in range(2)]
        kT = [pp.tile([128, C], BF16, name=f"kT{p}") for p in range(2)]
        v = pp.tile([128, CC * HL * DH], BF16, name="v")
        wo = [pp.tile([128, D], BF16, name=f"wo{p}") for p in range(2)]
        msk_sb = pp.tile([65, 128], BF16, name="msk_sb")
        iden_sb = pp.tile([128, 128], BF16, name="iden_sb")
        b_sb = [pp.tile([128, 3], F32, name=f"b{p}") for p in range(2)]
        # ones column: stationary operand of the softmax-denominator matmuls
        ones_col = pp.tile([128, 1], BF16, name="ones_col")
        nc.gpsimd.memset(ones_col, 1.0)

        # All weight chunks ride ONE jumbo DMA descriptor into a single
        # [128, KC*768] tile (k-chunk k at free offset k*768): one trigger
        # instead of 8 on the head-critical sync queue.
        w_all = wp.tile([128, KC * 3 * DL], BF16, name="w_all")
        w_sb = [w_all[:, ds(k * 3 * DL, 3 * DL)] for k in range(KC)]
        xs = [sp.tile([128, T], BF16, tag=f"x{k}", name=f"xs{k}") for k in range(KC)]
        cs = [sp.tile([128, C], BF16, tag=f"c{k}", name=f"cs{k}") for k in range(KC)]

        # Priority order on the sync HWDGE queue: w (one jumbo trigger) then
        # x then ctx — everything the projections block on, in need-order so
        # transfers don't compete with each other for HBM bandwidth. The
        # scalar HWDGE queue carries only the small constants concurrently.
        nc.sync.dma_start(
            out=w_all.rearrange("p (k c) -> p k c", k=KC),
            in_=wqkvT.rearrange("(k p) c -> p k c", k=KC),
        )
        for k in range(KC):
            nc.sync.dma_start(out=xs[k], in_=xT[ts(k, 128), :])
        for k in range(KC):
            nc.sync.dma_start(out=cs[k], in_=ctxT[ts(k, 128), :])
        nc.scalar.dma_start(out=iden_sb, in_=iden)
        for p in range(2):
            nc.scalar.dma_start(out=b_sb[p], in_=bqkv[ts(p, 128), :])
        nc.scalar.dma_start(out=msk_sb, in_=msk)
        for p in range(2):
            nc.scalar.dma_start(out=wo[p], in_=woT[ts(p, 128), :])

        bq_sb = [b_sb[p][:, 0:1] for p in range(2)]
        bk_sb = [b_sb[p][:, 1:2] for p in range(2)]
        bv_sb = [b_sb[p][:, 2:3] for p in range(2)]

        wq = [w_sb[k][:, 0:DL] for k in range(KC)]
        wk = [w_sb[k][:, DL : 2 * DL] for k in range(KC)]
        wv = [w_sb[k][:, 2 * DL : 3 * DL] for k in range(KC)]

        # ---------------- Phase A: Q and V projections ----------------
        # k-outer loops: all output PSUM groups live at once, so matmuls
        # issue as each input k-chunk's DMA lands instead of waiting for the
        # full tensor (PE saturated from the first chunk arrival).
        with tc.psum_pool(name="qpp", bufs=8) as qpp:
            qgr = [
                qpp.tile([128, 512], F32, tag="proj", name=f"qp{p}{tq}")
                for p in range(2)
                for tq in range(4)
            ]
            for k in range(KC):
                for i in range(8):
                    p, tq = i // 4, i % 4
                    nc.tensor.matmul(
                        qgr[i],
                        lhsT=wq[k][:, ts(p, 128)],
                        rhs=xs[k][:, ts(tq, 512)],
                        start=(k == 0),
                        stop=(k == KC - 1),
                    )
            for i in range(8):
                p, tq = i // 4, i % 4
                nc.vector.tensor_scalar_add(
                    out=qT[p][:, ts(tq, 512)], in0=qgr[i], scalar1=bq_sb[p]
                )

        # V projection + PE transposes (K is interleaved into attention).
        # v layout: [c=128, (ci, h) * 64] — 64 cols per head, heads of one
        # p-group adjacent, so the transposed [128,128] block lands as one
        # contiguous copy.
        with (
            tc.sbuf_pool(name="vstage", bufs=4) as vp,
            tc.psum_pool(name="pps", bufs=6) as pps,
            tc.psum_pool(name="tps", bufs=2) as tps,
        ):
            for p in range(2):
                vgr = [
                    pps.tile([128, 512], F32, tag="proj", name=f"vp{p}{cq}")
                    for cq in range(6)
                ]
                for k in range(KC):
                    for cq in range(6):
                        nc.tensor.matmul(
                            vgr[cq],
                            lhsT=wv[k][:, ts(p, 128)],
                            rhs=cs[k][:, ts(cq, 512)],
                            start=(k == 0),
                            stop=(k == KC - 1),
                        )
                for cq in range(6):
                    vst = vp.tile([128, 512], BF16, tag="vst", name=f"vs{p}{cq}")
                    nc.vector.tensor_scalar_add(out=vst, in0=vgr[cq], scalar1=bv_sb[p])
                    for cb in range(4):
                        ci = cq * 4 + cb
                        tp_ = tps.tile([128, 128], BF16, tag="tr", name=f"tr{ci}{p}")
                        nc.tensor.transpose(tp_, vst[:, ts(cb, 128)], iden_sb)
                        nc.vector.tensor_copy(
                            out=v[:, ds((ci * HL + 2 * p) * DH, 2 * DH)], in_=tp_
                        )

        # ---------------- Phases B + C (K-proj + out-proj interleaved) ----
        with (
            tc.sbuf_pool(name="uop", bufs=1) as up,
            tc.sbuf_pool(name="rsp", bufs=1) as rp,
            tc.sbuf_pool(name="obp", bufs=3) as ob,
            tc.sbuf_pool(name="expp", bufs=4) as ep,
            tc.sbuf_pool(name="rcp", bufs=2) as rc,
            tc.psum_pool(name="qkps", bufs=2) as qps,
            tc.psum_pool(name="uops", bufs=1) as ups,
        ):
            uo = [up.tile([128, T], BF16, name=f"uo{pr}") for pr in range(2)]
            rs = [rp.tile([65, T], BF16, name=f"rs{pr}") for pr in range(2)]
            for pr in range(2):
                nc.gpsimd.memset(rs[pr], 1.0)

            def kproj_block(pp_, cq):
                def emit():
                    ps = qps.tile([128, 512], F32, tag="qk", name=f"kp{pp_}{cq}")
                    for k in range(KC):
                        nc.tensor.matmul(
                            ps,
                            lhsT=wk[k][:, ts(pp_, 128)],
                            rhs=cs[k][:, ts(cq, 512)],
                            start=(k == 0),
                            stop=(k == KC - 1),
                        )
                    nc.vector.tensor_scalar_add(
                        out=kT[pp_][:, ts(cq, 512)], in0=ps, scalar1=bk_sb[pp_]
                    )

                return emit

            def c_tile_half(tt, dc, osb_holder, on_act=False):
                # half an out-projection t-tile (one 512-wide dc chunk);
                # split so a single filler invocation stays under the per-ci
                # PE headroom while ACT streams the exp.
                def emit():
                    if dc == 0:
                        osb_holder["t"] = ob.tile(
                            [128, D], BF16, tag="ob", name=f"ob{tt}"
                        )
                    osb = osb_holder["t"]
                    o_ps = qps.tile([128, 512], F32, tag="qk", name=f"o{tt}_{dc}")
                    for p in range(2):
                        nc.tensor.matmul(
                            o_ps,
                            lhsT=uo[p][:, ts(tt, 128)],
                            rhs=wo[p][:, ts(dc, 512)],
                            start=(p == 0),
                            stop=(p == 1),
                        )
                    if on_act and dc == 1:
                        nc.scalar.copy(out=osb[:, ts(dc, 512)], in_=o_ps)
                    else:
                        nc.vector.tensor_copy(out=osb[:, ts(dc, 512)], in_=o_ps)
                    if dc == 1:
                        nc.sync.dma_start(out=out[ts(tt, 128), :], in_=osb)

                return emit

            def c_tile(tt, on_act=False):
                h = {}
                a = c_tile_half(tt, 0, h, on_act)
                b = c_tile_half(tt, 1, h, on_act)

                def emit():
                    a()
                    b()

                return emit

            def norm_half(pr, th, tq):
                # broadcast denominators -> [128, 512], recip, scale uo half
                def emit():
                    off = th * 1024 + tq * 512
                    bc = qps.tile([128, 512], F32, tag="qk", name=f"bc{pr}{th}{tq}")
                    nc.tensor.matmul(
                        bc,
                        lhsT=msk_sb,
                        rhs=rs[pr][:, ds(off, 512)],
                        start=True,
                        stop=True,
                    )
                    rcl = rc.tile([128, 512], F32, tag="rc", name=f"rcl{pr}{th}{tq}")
                    nc.vector.reciprocal_approx_fast(rcl, bc)
                    nc.vector.tensor_mul(
                        out=uo[pr][:, ds(off, 512)],
                        in0=uo[pr][:, ds(off, 512)],
                        in1=rcl,
                    )

                return emit

            def attn_pair(p, tb, pre, post):
                # heads (2p, 2p+1) together. QK: the two heads' matmuls are
                # row-tiled at PE positions (0,0)/(64,0) and issued ABAB —
                # measured on hw, adjacent pairs stream concurrently. PV: the
                # two heads are column-tiled at (0,0)/(0,64) into ONE
                # [128,1024] accumulator (h0 rows 0-63, h1 rows 64-127), plus
                # a column-tiled pair of ones-matmuls accumulating the softmax
                # denominators at partitions 0/64 of den_ps. Per ci: QK + exp
                # issue first (keeps ACT fed), then the previous ci's PV+den,
                # then fillers riding in the PE headroom under the ACT slot.
                uo_ps = ups.tile([128, 1024], F32, tag="uo", name=f"up{p}{tb}")
                den_ps = ups.tile([65, 1024], F32, tag="den", name=f"dn{p}{tb}")

                def pv_pair(ci, exs):
                    for tq in range(2):
                        for h2 in range(2):
                            h = 2 * p + h2
                            nc.tensor.matmul(
                                uo_ps[ds(64 * h2, 64), ts(tq, 512)],
                                lhsT=v[:, ds((ci * HL + h) * DH, DH)],
                                rhs=exs[h2][:, ts(tq, 512)],
                                start=(ci == 0),
                                stop=(ci == CC - 1),
                                tile_position=(0, 64 * h2),
                            )
                        for h2 in range(2):
                            nc.tensor.matmul(
                                den_ps[ds(64 * h2, 1), ts(tq, 512)],
                                lhsT=ones_col,
                                rhs=exs[h2][:, ts(tq, 512)],
                                start=(ci == 0),
                                stop=(ci == CC - 1),
                                tile_position=(0, 64 * h2),
                            )

                pend = deque()
                for ci in range(CC):
                    for f in pre.get(ci, ()):
                        f()
                    qks = [
                        qps.tile([128, 1024], F32, tag="qk", name=f"qk{p}{tb}_{ci}_{h2}")
                        for h2 in range(2)
                    ]
                    for tq in range(2):
                        for h2 in range(2):
                            base = 64 * h2
                            nc.tensor.matmul(
                                qks[h2][:, ts(tq, 512)],
                                lhsT=kT[p][ds(base, 64), ts(ci, 128)],
                                rhs=qT[p][ds(base, 64), ds(tb * 1024 + tq * 512, 512)],
                                start=True,
                                stop=True,
                                tile_position=(base, 0),
                            )
                    exs = []
                    for h2 in range(2):
                        ex = ep.tile(
                            [128, 1024], BF16, tag="exp", name=f"ex{p}{tb}_{ci}_{h2}"
                        )
                        nc.scalar.activation(ex, qks[h2], AF.Exp, scale=0.125)
                        exs.append(ex)
                    while pend:
                        pv_pair(*pend.popleft())
                    pend.append((ci, exs))
                    for f in post.get(ci, ()):
                        f()
                # tail: last ci's PV + drain (DVE; ACT stays on the exp
                # stream). uo_ps is already in the packed [2*64, t] layout
                # of uo, so the drain is one [128,1024] copy.
                while pend:
                    pv_pair(*pend.popleft())

                def drain():
                    nc.vector.tensor_copy(
                        out=uo[p][:, ds(tb * 1024, 1024)], in_=uo_ps
                    )
                    for h2 in range(2):
                        nc.vector.tensor_copy(
                            out=rs[p][ds(64 * h2, 1), ds(tb * 1024, 1024)],
                            in_=den_ps[ds(64 * h2, 1), :],
                        )

                return drain

            # Each pair's drain is emitted at the pair boundary (DVE, off the
            # ACT critical path); norms run as fillers inside the NEXT pair.
            # K-proj feeds pairs 1-2 just-in-time, one cq block ahead;
            # out-projection t-tiles 0-7 hide in pairs 3-4 as split halves.
            # pair 1 (p0,tb0): only ci0 uses pre-fillers (kT cq0-1 gate QK)
            dr = attn_pair(
                0,
                0,
                {0: [kproj_block(0, 0), kproj_block(0, 1)]},
                {
                    1: [kproj_block(0, 2)],
                    5: [kproj_block(0, 3)],
                    9: [kproj_block(0, 4)],
                    13: [kproj_block(0, 5)],
                    17: [kproj_block(1, 0)],
                    20: [kproj_block(1, 1)],
                    22: [kproj_block(1, 2)],
                },
            )
            dr()
            # pair 2 (p1,tb0)
            dr = attn_pair(
                1,
                0,
                {},
                {
                    2: [kproj_block(1, 3)],
                    4: [norm_half(0, 0, 0)],
                    6: [norm_half(0, 0, 1)],
                    9: [kproj_block(1, 4)],
                    14: [kproj_block(1, 5)],
                },
            )
            dr()
            ct_h = [dict() for _ in range(TT)]
            # pair 3 (p1,tb1)
            dr = attn_pair(
                1,
                1,
                {},
                {
                    2: [norm_half(1, 0, 0)],
                    3: [norm_half(1, 0, 1)],
                    4: [c_tile_half(0, 0, ct_h[0])],
                    6: [c_tile_half(0, 1, ct_h[0])],
                    9: [c_tile_half(1, 0, ct_h[1])],
                    11: [c_tile_half(1, 1, ct_h[1])],
                    14: [c_tile_half(2, 0, ct_h[2])],
                    16: [c_tile_half(2, 1, ct_h[2])],
                    19: [c_tile_half(3, 0, ct_h[3])],
                    21: [c_tile_half(3, 1, ct_h[3])],
                },
            )
            dr()
            # pair 4 (p0,tb1)
            dr = attn_pair(
                0,
                1,
                {},
                {
                    2: [norm_half(1, 1, 0)],
                    3: [norm_half(1, 1, 1)],
                    4: [c_tile_half(4, 0, ct_h[4])],
                    6: [c_tile_half(4, 1, ct_h[4])],
                    9: [c_tile_half(5, 0, ct_h[5])],
                    11: [c_tile_half(5, 1, ct_h[5])],
                    14: [c_tile_half(6, 0, ct_h[6])],
                    16: [c_tile_half(6, 1, ct_h[6])],
                    19: [c_tile_half(7, 0, ct_h[7])],
                    21: [c_tile_half(7, 1, ct_h[7])],
                },
            )
            dr()
            norm_half(0, 1, 0)()
            norm_half(0, 1, 1)()
            for tt in range(TT // 2, TT):
                c_tile(tt, on_act=True)()


def _build_nc():
    nc = bacc.Bacc("TRN2", target_bir_lowering=False, debug=False, num_devices=NCORES)
    xT = nc.dram_tensor("xT", [D, T], BF16, kind="ExternalInput").ap()
    ctxT = nc.dram_tensor("ctxT", [D, C], BF16, kind="ExternalInput").ap()
    wqkvT = nc.dram_tensor("wqkvT", [D, 3 * DL], BF16, kind="ExternalInput").ap()
    woT = nc.dram_tensor("woT", [DL, D], BF16, kind="ExternalInput").ap()
    bqkv = nc.dram_tensor("bqkv", [DL, 3], F32, kind="ExternalInput").ap()
    msk = nc.dram_tensor("msk", [65, 128], BF16, kind="ExternalInput").ap()
    iden = nc.dram_tensor("iden", [128, 128], BF16, kind="ExternalInput").ap()
    out = nc.dram_tensor("out", [T, D], BF16, kind="ExternalOutput").ap()
    with tile.TileContext(nc) as tc:
        _emit(nc, tc, (xT, ctxT, wqkvT, woT, bqkv, msk, iden, out))
    nc.compile()
    return nc


_NC_CACHE = None


def _get_nc():
    global _NC_CACHE
    if _NC_CACHE is None:
        _NC_CACHE = _build_nc()
    return _NC_CACHE


def _make_in_maps(inputs):
    x = np.asarray(inputs["x"], dtype=np.float32)
    context = np.asarray(inputs["context"], dtype=np.float32)
    Wq = np.asarray(inputs["Wq"], dtype=np.float32)
    Wk = np.asarray(inputs["Wk"], dtype=np.float32)
    Wv = np.asarray(inputs["Wv"], dtype=np.float32)
    Wo = np.asarray(inputs["Wo"], dtype=np.float32)
    bq = np.asarray(inputs["bq"], dtype=np.float32)
    bk = np.asarray(inputs["bk"], dtype=np.float32)
    bv = np.asarray(inputs["bv"], dtype=np.float32)

    msk = np.zeros((65, 128), _bf16)
    msk[0, :64] = 1.0
    msk[64, 64:] = 1.0
    iden = np.eye(128, dtype=_bf16)

    xTs = [np.ascontiguousarray(x[b].T).astype(_bf16) for b in range(B)]
    cTs = [np.ascontiguousarray(context[b].T).astype(_bf16) for b in range(B)]

    in_maps = []
    for core in range(NCORES):
        b, hg = core // 4, core % 4
        sl = slice(hg * DL, (hg + 1) * DL)
        in_maps.append(
            {
                "xT": xTs[b],
                "ctxT": cTs[b],
                "wqkvT": np.ascontiguousarray(
                    np.concatenate([Wq[sl].T, Wk[sl].T, Wv[sl].T], axis=1)
                ).astype(_bf16),
                "woT": np.ascontiguousarray(Wo[:, sl].T).astype(_bf16),
                "bqkv": np.ascontiguousarray(
                    np.stack([bq[sl], bk[sl], bv[sl]], axis=1)
                ),
                "msk": msk,
                "iden": iden,
            }
        )
    return in_maps


def run_spmd(inputs, trace=False):
    """Run the SPMD kernel; returns (full output [B,T,D], BassKernelResults)."""
    in_maps = _make_in_maps(inputs)
    res = run_bass_kernel_spmd(
        _get_nc(), in_maps, core_ids=list(range(NCORES)), trace=trace
    )
    bo = np.asarray(inputs["bo"], dtype=np.float32)
    y = np.zeros((B, T, D), np.float32)
    for core in range(NCORES):
        y[core // 4] += np.asarray(res.results[core]["out"], dtype=np.float32)
    y += bo.reshape(1, 1, D)
    return y, res


def kernel(**inputs):
    y, _ = run_spmd(inputs, trace=False)
    return y
